# revision 43
# baseline (speedup 1.0000x reference)
"""Trainium2 Bass kernel for a 6-layer post-LN Transformer encoder.

Strategy (8 NeuronCores):
  - Interleaved sequence-parallel: core i owns tokens [i*256,(i+1)*256) of
    BOTH batches (columns [0:256]=batch0, [256:512]=batch1). All per-token ops
    (projections, LN, FFN) are batch-agnostic; attention splits into
    per-batch halves.
  - Per layer, K^T and V are quantized to fp8e4 and AllGathered across all 8
    cores. Projection order K -> AG(K) -> V -> AG(V) -> Q so both collectives
    overlap projection compute.
  - Activations kept feature-major ([D, tok]); LayerNorm statistics via bf16
    ones-matmuls on pre-LN bf16 copies (cast on the Scalar engine, squares on
    Vector) accumulated into a reused PSUM slab; the rsqrt scale is
    Sqrt (ScalarE) + reciprocal_approx_fast (DVE) at full partition width so
    no broadcast matmul / Ln+Exp table thrash is needed; dummy activations
    preload ACT table sets off the critical path.
    (fp8 GEMM inputs for the projections/FFN were tried and reverted: each
    fp8xfp8 GEMM adds ~5% incoherent-sum noise -> ~10% final, over budget.)
  - Softmax: scores transposed (s^T[k, q]); exp() on the Scalar engine;
    denominator via a ones-column appended to V in the attn@V matmul, then
    reciprocal_approx_fast + a one-hot selector matmul to broadcast 1/den
    back over head dims (no DRAM round-trip).
"""

import numpy as np
import ml_dtypes

L, D, H, FF = 6, 1024, 16, 4096
DK = D // H          # 64
B, S = 2, 2048
NCORES = 8
TPB = S // NCORES    # 256 tokens per batch per core
T = 2 * TPB          # 512 local tokens per core (256 b0 + 256 b1)
DC = D // 128        # 8
FC = FF // 128       # 32
KC = S // 128        # 16 key chunks per batch
TC = T // 128        # 4
EPS = 1e-5
BF16 = ml_dtypes.bfloat16
FP8 = ml_dtypes.float8_e4m3
WS = 32.0            # fp8 weight pre-scale (host)
XS = 8.0             # fp8 activation pre-scale (on-chip)

_CACHE = {}


def _build_nc():
    import contextlib
    import concourse.bacc as bacc
    import concourse.mybir as mybir
    import concourse.tile as tile
    import concourse.bass as bass
    from concourse.bass import ts, ds

    f32 = mybir.dt.float32
    bf16 = mybir.dt.bfloat16
    fp8 = mybir.dt.float8e4
    AF = mybir.ActivationFunctionType
    OP = mybir.AluOpType
    DR = mybir.MatmulPerfMode.DoubleRow
    WXS = float(WS * XS)  # combined weight x activation fp8 pre-scale

    nc = bacc.Bacc(num_devices=NCORES)

    # ---- parameters -----------------------------------------------------
    x0T = nc.declare_dram_parameter("x0T", [D, T], f32, isOutput=False)
    wq = nc.declare_dram_parameter("wq", [L, D, D], bf16, isOutput=False)
    wk = nc.declare_dram_parameter("wk", [L, D, D], bf16, isOutput=False)
    wv = nc.declare_dram_parameter("wv", [L, D, D], bf16, isOutput=False)
    wo = nc.declare_dram_parameter("wo", [L, D, D], bf16, isOutput=False)
    w1 = nc.declare_dram_parameter("w1", [L, D, FF], bf16, isOutput=False)
    # W2 pre-packed on host: [L, mc(8), o(32), p(128), m(128)]
    w2p = nc.declare_dram_parameter("w2p", [L, DC, FC, 128, 128], bf16, isOutput=False)
    bq = nc.declare_dram_parameter("bq", [L, D], f32, isOutput=False)
    bk = nc.declare_dram_parameter("bk", [L, D], f32, isOutput=False)
    bvb = nc.declare_dram_parameter("bvb", [L, D], bf16, isOutput=False)
    bo = nc.declare_dram_parameter("bo", [L, D], f32, isOutput=False)
    b1 = nc.declare_dram_parameter("b1", [L, FF], f32, isOutput=False)
    b2 = nc.declare_dram_parameter("b2", [L, D], f32, isOutput=False)
    g1 = nc.declare_dram_parameter("g1", [L, D], f32, isOutput=False)
    be1 = nc.declare_dram_parameter("be1", [L, D], f32, isOutput=False)
    g2 = nc.declare_dram_parameter("g2", [L, D], f32, isOutput=False)
    be2 = nc.declare_dram_parameter("be2", [L, D], f32, isOutput=False)
    sel16p = nc.declare_dram_parameter("sel16p", [H, DC * 128], bf16,
                                       isOutput=False)
    outT = nc.declare_dram_parameter("outT", [D, T], f32, isOutput=True)

    groups8 = [list(range(NCORES))]

    with tile.TileContext(nc) as tc:
        ctx = contextlib.ExitStack()
        singles = ctx.enter_context(tc.tile_pool(name="singles", bufs=1))
        params = ctx.enter_context(tc.tile_pool(name="params", bufs=2))
        wpool = ctx.enter_context(tc.tile_pool(name="wpool", bufs=2))
        w1pool = ctx.enter_context(tc.tile_pool(name="w1pool", bufs=2))
        w2pool = ctx.enter_context(tc.tile_pool(name="w2pool", bufs=2))
        kgpool = ctx.enter_context(tc.tile_pool(name="kgpool", bufs=2))
        vgpool = ctx.enter_context(tc.tile_pool(name="vgpool", bufs=1))
        vgppool = ctx.enter_context(tc.tile_pool(name="vgppool", bufs=2))
        apool = ctx.enter_context(tc.tile_pool(name="apool", bufs=16))
        xbfpool = ctx.enter_context(tc.tile_pool(name="xbfpool", bufs=2))
        xprepool = ctx.enter_context(tc.tile_pool(name="xprepool", bufs=1))
        qkvpool = ctx.enter_context(tc.tile_pool(name="qkvpool", bufs=1))
        anfpool = ctx.enter_context(tc.tile_pool(name="anfpool", bufs=8))
        ao65pool = ctx.enter_context(tc.tile_pool(name="ao65pool", bufs=2))
        tmp = ctx.enter_context(tc.tile_pool(name="tmp", bufs=1))
        denp = ctx.enter_context(tc.tile_pool(name="denp", bufs=1))
        dram = ctx.enter_context(tc.tile_pool(name="dram", bufs=2, space="DRAM"))
        pscore = ctx.enter_context(tc.tile_pool(name="pscore", bufs=2, space="PSUM"))
        pav = ctx.enter_context(tc.tile_pool(name="pav", bufs=2, space="PSUM"))
        pmisc = ctx.enter_context(tc.tile_pool(name="pmisc", bufs=2, space="PSUM"))

        # ---- constants + resident state --------------------------------
        xT = singles.tile([128, DC, T], f32, name="xT")
        nc.sync.dma_start(out=xT, in_=x0T[:, :].rearrange("(c p) t -> p c t", p=128))
        ones_row_bf = singles.tile([1, 128], bf16, name="ones_row_bf")
        nc.vector.memset(ones_row_bf, 1.0)
        eps_col = singles.tile([128, 1], f32, name="eps_col")
        nc.vector.memset(eps_col, EPS)
        onesD_bf = singles.tile([128, 128], bf16, name="onesD_bf")
        nc.vector.memset(onesD_bf, 1.0 / D)
        # one-hot selector: sel16[h, j*128 + (h%2)*64 + d] = 1 iff h//2 == j
        sel16 = singles.tile([16, DC * 128], bf16, name="sel16")
        nc.sync.dma_start(out=sel16, in_=sel16p[:, :])
        junk = singles.tile([1, 2], f32, name="junk")
        xbf_cur = xbfpool.tile([128, DC, T], bf16, tag="xbf")
        for c in range(DC):
            nc.vector.tensor_copy(xbf_cur[:, c, :], xT[:, c, :])

        def emit_output_chunk(mc, ps, bias_col, stats_ps):
            """xT[mc] += ps + bias; stage stats for the next LN."""
            nc.vector.scalar_tensor_tensor(xT[:, mc, :], ps,
                                           bias_col, xT[:, mc, :],
                                           OP.add, OP.add)
            xpre = xprepool.tile([128, 512], bf16, tag="xpre", bufs=2, name="xpre")
            nc.scalar.copy(xpre, xT[:, mc, :])
            sq = xprepool.tile([128, 512], bf16, tag="sqb", bufs=2, name="sq")
            nc.vector.tensor_tensor(sq, xpre, xpre, OP.mult)
            nc.tensor.matmul(stats_ps[:, 0:512], onesD_bf, xpre,
                             start=(mc == 0), stop=(mc == DC - 1))
            nc.tensor.matmul(stats_ps[:, 512:1024], onesD_bf, sq,
                             start=(mc == 0), stop=(mc == DC - 1))

        def ln_finish(stats_ps, g_sb, be_sb, xbf_out):
            """x = LN(x) in place from stats; writes bf16 copy into xbf_out."""
            msq = tmp.tile([128, 512], f32, tag="tA", name="msq")
            nc.scalar.square(msq, stats_ps[:, 0:512])
            e2 = tmp.tile([128, 512], f32, tag="tB", name="e2")
            nc.vector.tensor_tensor(e2, stats_ps[:, 512:1024], msq, OP.subtract)
            sdv = tmp.tile([128, 512], f32, tag="tA", name="sdv")
            nc.scalar.activation(sdv, e2, AF.Sqrt, bias=eps_col)
            # preload the exp table set off the critical path (next exps /
            # relus are all servable by it; Copy/Square are in every set)
            nc.scalar.activation(junk[0:1, 0:1], eps_col[0:1, 0:1], AF.Exp)
            s_bc = tmp.tile([128, 512], f32, tag="tB", name="s_bc")
            with nc.allow_low_precision(reason="LN rsqrt, 18-bit recip ok"):
                nc.vector.reciprocal_approx_fast(out=s_bc, in_=sdv)
            for c in range(DC):
                t1 = tmp.tile([128, 512], f32, tag="t1", bufs=2, name="t1")
                nc.vector.tensor_tensor(t1, xT[:, c, :], stats_ps[:, 0:512],
                                        OP.subtract)
                nc.vector.tensor_tensor(t1, t1, s_bc, OP.mult)
                nc.vector.tensor_scalar(xT[:, c, :], t1, g_sb[:, c:c + 1],
                                        be_sb[:, c:c + 1], OP.mult, OP.add)
                nc.scalar.copy(xbf_out[:, c, :], xT[:, c, :])

        for l in range(L):
            # ---- per-layer params --------------------------------------
            pp = params.tile([128, 8, DC], f32, tag="pcol")
            for i_, t_src in enumerate([bq, bk, bo, b2, g1, be1, g2, be2]):
                nc.sync.dma_start(out=pp[:, i_, :],
                                  in_=t_src[l].rearrange("(c p) -> p c", p=128))
            bq_sb, bk_sb, bo_sb, b2_sb = pp[:, 0], pp[:, 1], pp[:, 2], pp[:, 3]
            g1_sb, be1_sb, g2_sb, be2_sb = pp[:, 4], pp[:, 5], pp[:, 6], pp[:, 7]
            b1_sb = params.tile([128, FC], f32, tag="pc32")
            nc.sync.dma_start(out=b1_sb, in_=b1[l].rearrange("(c p) -> p c", p=128))
            bv_row = params.tile([1, D], bf16, tag="bv_row", bufs=1)
            nc.sync.dma_start(out=bv_row, in_=bvb[l][None, :])

            xbf = xbf_cur

            # DRAM staging + gathered buffers (Shared, 8-rank); K is
            # gathered in halves so scores can start before V's gather and
            # the CC engine pipeline stays busy: order K-h1 -> V -> K-h2.
            kt_loc = dram.tile([D, T], fp8, tag="kt_loc")
            kt_g = dram.tile([NCORES * D, T], fp8, tag="kt_g",
                             addr_space="Shared")
            v_loc = dram.tile([T, D], fp8, tag="v_loc")
            v_g = dram.tile([NCORES * T, D], fp8, tag="v_g",
                            addr_space="Shared")

            kt_sb = qkvpool.tile([128, DC, T], fp8, tag="kt")

            # ---- K projection (fp8 DoubleRow) + single AllGather -------
            for kh in range(2):
                wk_h = wpool.tile([128, DC, 512], bf16, tag="w", name=f"wk_h{kh}")
                nc.sync.dma_start(
                    out=wk_h,
                    in_=wk[l][:, ds(kh * 512, 512)].rearrange("(c p) m -> p c m", p=128))
                for mcb in range(4):
                    mc = kh * 4 + mcb
                    ps = pmisc.tile([128, 512], f32, tag="pmisc")
                    for c in range(DC):
                        nc.tensor.matmul(ps, wk_h[:, c, ts(mcb, 128)], xbf[:, c, :],
                                         start=(c == 0), stop=(c == DC - 1))
                    nc.vector.tensor_scalar(kt_sb[:, mc, :], ps,
                                            bk_sb[:, mc:mc + 1], None, OP.add)
            nc.gpsimd.dma_start(
                out=kt_loc[:, :].rearrange("(c p) t -> p c t", p=128),
                in_=kt_sb)
            nc.gpsimd.collective_compute(
                "AllGather", OP.bypass, replica_groups=groups8,
                ins=[kt_loc.opt()], outs=[kt_g.opt()])

            # ---- V projection (fp8 DoubleRow) + AllGather --------------
            # V weights stage in a w1pool slot (idle until the FFN1 prefetch)
            # so the wpool rotation lets wq's DMA start right after K proj
            # releases its buffer, instead of stalling behind wv.
            v_sb = qkvpool.tile([128, TC, D], fp8, tag="v")
            wv_b = w1pool.tile([128, DC, 1024], bf16, tag="w1b", name="wv_b")
            for nh in range(2):
                nc.sync.dma_start(
                    out=wv_b[:, :, ds(nh * 512, 512)],
                    in_=wv[l][:, ds(nh * 512, 512)].rearrange("(c p) m -> p c m", p=128))
                for t_ in range(TC):
                    ps = pmisc.tile([128, 512], f32, tag="pmisc")
                    for c in range(DC):
                        nc.tensor.matmul(ps, xbf[:, c, ts(t_, 128)],
                                         wv_b[:, c, ds(nh * 512, 512)],
                                         start=(c == 0), stop=False)
                    nc.tensor.matmul(ps, ones_row_bf, bv_row[:, ds(nh * 512, 512)],
                                     start=False, stop=True)
                    nc.vector.tensor_copy(v_sb[:, t_, ds(nh * 512, 512)], ps)
            nc.gpsimd.dma_start(out=v_loc[:, :].rearrange("(c p) d -> p c d", p=128),
                                in_=v_sb)
            nc.gpsimd.collective_compute(
                "AllGather", OP.bypass, replica_groups=groups8,
                ins=[v_loc.opt()], outs=[v_g.opt()])

            # ---- Q projection (overlaps both AllGathers) ---------------
            qT = qkvpool.tile([128, DC, T], bf16, tag="qT")
            for qh in range(2):
                wq_h = wpool.tile([128, DC, 512], bf16, tag="w", name=f"wq_h{qh}")
                nc.sync.dma_start(
                    out=wq_h,
                    in_=wq[l][:, ds(qh * 512, 512)].rearrange("(c p) m -> p c m", p=128))
                for mcb in range(4):
                    mc = qh * 4 + mcb
                    ps = pmisc.tile([128, 512], f32, tag="pmisc")
                    for c in range(DC):
                        nc.tensor.matmul(ps, wq_h[:, c, ts(mcb, 128)], xbf[:, c, :],
                                         start=(c == 0), stop=(c == DC - 1))
                    nc.vector.tensor_scalar(qT[:, mc, :], ps,
                                            bq_sb[:, mc:mc + 1], None, OP.add)

            # prefetch Wo + first W1 halves during attention
            wo_h1 = wpool.tile([128, DC, 512], bf16, tag="w", name="wo_h1")
            nc.sync.dma_start(out=wo_h1,
                              in_=wo[l][:, 0:512].rearrange("(c p) m -> p c m", p=128))
            wo_h2 = wpool.tile([128, DC, 512], bf16, tag="w", name="wo_h2")
            nc.sync.dma_start(out=wo_h2,
                              in_=wo[l][:, 512:1024].rearrange("(c p) m -> p c m", p=128))
            w1_tiles = {}
            for b4 in range(2):
                w1b = w1pool.tile([128, DC, 1024], bf16, tag="w1b",
                                  name=f"w1b{b4}")
                w1_tiles[b4] = w1b
                nc.sync.dma_start(
                    out=w1b,
                    in_=w1[l][:, ds(b4 * 1024, 1024)]
                        .rearrange("(c p) m -> p c m", p=128))

            # ---- attention ---------------------------------------------
            # gathered layouts (all-static reads; every core needs both batches):
            #   kt_g rows: r(8) x pairchunk(4) x p(128); cols: [b0 256 | b1 256]
            #   v_g  rows: c32(32) x p(128) = rank r tokens [b0 256|b1 256]; cols D
            # batch-b key chunk kc (128 keys): rank r=kc//2, col c4 = 2*b+(kc%2)
            # v_g row chunk for (b, kc): c32 = (kc//2)*4 + 2*b + (kc%2)
            den_sb = denp.tile([16, 512], bf16, tag="den", name="den_sb")
            an_bf = xbfpool.tile([128, DC, T], bf16, tag="xbf")
            anf_tiles = []
            for j in range(DC):  # head pairs (2j, 2j+1)
                kgj = kgpool.tile([128, NCORES, T], fp8, tag="kg")
                nc.gpsimd.dma_start(
                    out=kgj,
                    in_=kt_g[:, :].rearrange("(r c p) t -> p r c t", p=128,
                                             c=8)[:, :, j, :])
                # per-pair V slice with ones columns: [128, 32, 130]
                vgp = vgppool.tile([128, 32, 130], fp8, tag="vgp")
                nc.gpsimd.dma_start(
                    out=vgp[:, :, 0:64],
                    in_=v_g[:, ds((2 * j) * 64, 64)]
                        .rearrange("(c p) w -> p c w", p=128))
                nc.gpsimd.dma_start(
                    out=vgp[:, :, 65:129],
                    in_=v_g[:, ds((2 * j + 1) * 64, 64)]
                        .rearrange("(c p) w -> p c w", p=128))
                nc.vector.memset(vgp[:, :, 64:65], 1.0)
                nc.vector.memset(vgp[:, :, 129:130], 1.0)

                at_tiles = []
                for kc in range(KC):
                    r, codd = kc // 2, kc % 2
                    pss = pscore.tile([128, 1024], f32, tag="pscore")
                    for b_ in range(2):
                        c4 = 2 * b_ + codd
                        nc.tensor.matmul(
                            pss[:, ds(b_ * 256, 256)],
                            kgj[0:64, r, ts(c4, 128)],
                            qT[0:64, j, ds(b_ * 256, 256)],
                            start=True, stop=True, tile_position=(0, 0))
                        nc.tensor.matmul(
                            pss[:, ds(512 + b_ * 256, 256)],
                            kgj[64:128, r, ts(c4, 128)],
                            qT[64:128, j, ds(b_ * 256, 256)],
                            start=True, stop=True, tile_position=(64, 0))
                    at = apool.tile([128, 1024], fp8, tag="attn")
                    nc.scalar.activation(at, pss, AF.Exp, scale=1.0 / 32.0)
                    at_tiles.append(at)
                anf = anfpool.tile([128, 512], bf16, tag="anf")
                for ab in range(2):
                    h = 2 * j + ab
                    for b_ in range(2):
                        pav_t = pav.tile([65, 512], f32, tag="pav")
                        for kc in range(KC):
                            c32 = (kc // 2) * 4 + 2 * b_ + (kc % 2)
                            nc.tensor.matmul(
                                pav_t[:, 0:256],
                                vgp[:, c32, ds(ab * 65, 65)],
                                at_tiles[kc][:, ds(ab * 512 + b_ * 256, 256)],
                                start=(kc == 0), stop=(kc == KC - 1))
                        ao65 = ao65pool.tile([65, 256], bf16, tag="ao65")
                        nc.vector.tensor_copy(ao65, pav_t[:, 0:256])
                        nc.sync.dma_start(
                            out=den_sb[h:h + 1, ds(b_ * 256, 256)],
                            in_=ao65[64:65, :])
                        nc.sync.dma_start(
                            out=anf[ds(ab * 64, 64), ds(b_ * 256, 256)],
                            in_=ao65[0:64, :])
                anf_tiles.append(anf)

            # preload the sqrt table set while attnV j=7 drains (off-path);
            # Copy/Square/Relu in between are servable by every set
            nc.scalar.activation(junk[0:1, 0:1], eps_col[0:1, 0:1], AF.Sqrt)

            # ---- normalize attn_out (1/den via approx recip + selector) -
            den_f_t = tmp.tile([128, 512], f32, tag="tA", name="den_f")
            den_f = den_f_t[0:16, :]
            nc.vector.tensor_copy(den_f, den_sb)
            den_r_t = tmp.tile([128, 512], f32, tag="tB", name="den_r")
            den_r = den_r_t[0:16, :]
            with nc.allow_low_precision(reason="softmax denom scale, ~0.4% ok"):
                nc.vector.reciprocal_approx_fast(out=den_r, in_=den_f)
            denr_bf = denp.tile([16, 512], bf16, tag="denrb", name="denr_bf")
            nc.vector.tensor_copy(denr_bf, den_r)
            for j in range(DC):
                rb_ps = pmisc.tile([128, 512], f32, tag="pmisc")
                nc.tensor.matmul(rb_ps, sel16[:, ts(j, 128)], denr_bf,
                                 start=True, stop=True)
                nc.vector.tensor_tensor(an_bf[:, j, :], anf_tiles[j], rb_ps,
                                        OP.mult)

            # ---- Wo + residual (+ LN1 stats staging) -------------------
            stats1 = pscore.tile([128, 1024], f32, tag="pscore", name="stats1")
            for mc in range(DC):
                wo_h = wo_h1 if mc < 4 else wo_h2
                ps = pmisc.tile([128, 512], f32, tag="pmisc")
                for c in range(DC):
                    nc.tensor.matmul(ps, wo_h[:, c, ts(mc % 4, 128)], an_bf[:, c, :],
                                     start=(c == 0), stop=(c == DC - 1))
                emit_output_chunk(mc, ps, bo_sb[:, mc:mc + 1], stats1)

            # ---- LN1 ----------------------------------------------------
            xbf2 = xbfpool.tile([128, DC, T], bf16, tag="xbf")
            ln_finish(stats1, g1_sb, be1_sb, xbf2)

            # ---- FFN ----------------------------------------------------
            ffT = vgpool.tile([128, FC, T], bf16, tag="vg")
            for b4 in range(4):
                if b4 + 2 < 4:
                    w1bn = w1pool.tile([128, DC, 1024], bf16, tag="w1b",
                                       name=f"w1b{b4 + 2}")
                    w1_tiles[b4 + 2] = w1bn
                    nc.sync.dma_start(
                        out=w1_tiles[b4 + 2],
                        in_=w1[l][:, ds((b4 + 2) * 1024, 1024)]
                            .rearrange("(c p) m -> p c m", p=128))
                w1b = w1_tiles[b4]
                for mcb in range(DC):
                    mc = b4 * 8 + mcb
                    ps = pmisc.tile([128, 512], f32, tag="pmisc")
                    for c in range(DC):
                        nc.tensor.matmul(ps, w1b[:, c, ts(mcb, 128)], xbf2[:, c, :],
                                         start=(c == 0), stop=(c == DC - 1))
                    nc.scalar.activation(ffT[:, mc, :], ps, AF.Relu,
                                         bias=b1_sb[:, mc:mc + 1])
            nc.scalar.activation(junk[0:1, 0:1], eps_col[0:1, 0:1], AF.Sqrt)
            stats2 = pscore.tile([128, 1024], f32, tag="pscore", name="stats2")
            for mc in range(DC):
                w2mc = w2pool.tile([128, FC, 128], bf16, tag="w2")
                nc.sync.dma_start(out=w2mc,
                                  in_=w2p[l, mc].rearrange("o p m -> p o m"))
                ps = pmisc.tile([128, 512], f32, tag="pmisc")
                for fc in range(FC):
                    nc.tensor.matmul(ps, w2mc[:, fc, :], ffT[:, fc, :],
                                     start=(fc == 0), stop=(fc == FC - 1))
                emit_output_chunk(mc, ps, b2_sb[:, mc:mc + 1], stats2)

            # ---- LN2 ----------------------------------------------------
            xbf_cur = xbfpool.tile([128, DC, T], bf16, tag="xbf")
            ln_finish(stats2, g2_sb, be2_sb, xbf_cur)

        # ---- output ----------------------------------------------------
        nc.sync.dma_start(out=outT[:, :].rearrange("(c p) t -> p c t", p=128), in_=xT)
        ctx.close()

    nc.compile()
    return nc


def _prepare_host(inputs):
    src = np.asarray(inputs["src"]).astype(np.int64)
    emb = np.asarray(inputs["emb"], dtype=np.float32)
    x = emb[src]                                    # [B, S, D] f32
    pos = np.arange(B, dtype=np.float32)[:, None]
    div = np.exp(np.arange(0, D, 2, dtype=np.float32) * (-np.log(10000.0) / D))
    pe = np.zeros((B, D), np.float32)
    pe[:, 0::2] = np.sin(pos / div)
    pe[:, 1::2] = np.cos(pos / div)
    x = x + pe[:, None, :]

    mask = np.asarray(inputs["src_mask"]).reshape(B, S)
    assert np.all(mask != 0), "kernel assumes all-ones src_mask (per problem spec)"

    f = np.float32
    shared = {
        "wq": np.ascontiguousarray(np.asarray(inputs["Wq"], f).astype(BF16)),
        "wk": np.ascontiguousarray(np.asarray(inputs["Wk"], f).astype(BF16)),
        "wv": np.ascontiguousarray(np.asarray(inputs["Wv"], f).astype(BF16)),
        "wo": np.ascontiguousarray(np.asarray(inputs["Wo"], f).astype(BF16)),
        "w1": np.ascontiguousarray(np.asarray(inputs["W1"], f).astype(BF16)),
        "w2p": np.ascontiguousarray(
            np.asarray(inputs["W2"], f).reshape(L, FC, 128, DC, 128)
            .transpose(0, 3, 1, 2, 4).astype(BF16)),
        "bq": np.ascontiguousarray(np.asarray(inputs["bq"], f)),
        "bk": np.ascontiguousarray(np.asarray(inputs["bk"], f)),
        "bvb": np.ascontiguousarray(np.asarray(inputs["bv"], f).astype(BF16)),
        "bo": np.ascontiguousarray(np.asarray(inputs["bo"], f)),
        "b1": np.ascontiguousarray(np.asarray(inputs["b1"], f)),
        "b2": np.ascontiguousarray(np.asarray(inputs["b2"], f)),
        "g1": np.ascontiguousarray(np.asarray(inputs["g1"], f)),
        "be1": np.ascontiguousarray(np.asarray(inputs["be1"], f)),
        "g2": np.ascontiguousarray(np.asarray(inputs["g2"], f)),
        "be2": np.ascontiguousarray(np.asarray(inputs["be2"], f)),
    }
    sel = np.zeros((H, DC * 128), np.float32)
    for h in range(H):
        o = (h // 2) * 128 + (h % 2) * 64
        sel[h, o:o + 64] = 1.0
    shared["sel16p"] = np.ascontiguousarray(sel.astype(BF16))
    in_maps = []
    for i in range(NCORES):
        t0 = i * TPB
        m = dict(shared)
        xi = np.concatenate([x[0, t0:t0 + TPB, :].T,
                             x[1, t0:t0 + TPB, :].T], axis=1)
        m["x0T"] = np.ascontiguousarray(xi.astype(np.float32))
        in_maps.append(m)
    return in_maps


def _run(in_maps, trace=False):
    from concourse.bass_utils import run_bass_kernel_spmd
    if "nc" not in _CACHE:
        _CACHE["nc"] = _build_nc()
    nc = _CACHE["nc"]
    res = run_bass_kernel_spmd(nc, in_maps, core_ids=list(range(NCORES)),
                               trace=trace)
    outs = res.results
    y = np.zeros((B, S, D), np.float32)
    for i in range(NCORES):
        t0 = i * TPB
        y[0, t0:t0 + TPB, :] = outs[i]["outT"][:, 0:TPB].T
        y[1, t0:t0 + TPB, :] = outs[i]["outT"][:, TPB:T].T
    return y, res


def kernel(**inputs) -> np.ndarray:
    in_maps = _prepare_host(inputs)
    y, _ = _run(in_maps, trace=False)
    return y


def kernel_traced(**inputs):
    """Same as kernel() but returns (output, BassKernelResults with profile)."""
    in_maps = _prepare_host(inputs)
    return _run(in_maps, trace=True)


# revision 45
# speedup vs baseline: 1.0081x; 1.0081x over previous
"""Trainium2 Bass kernel for a 6-layer post-LN Transformer encoder.

Strategy (8 NeuronCores):
  - Interleaved sequence-parallel: core i owns tokens [i*256,(i+1)*256) of
    BOTH batches (columns [0:256]=batch0, [256:512]=batch1). All per-token ops
    (projections, LN, FFN) are batch-agnostic; attention splits into
    per-batch halves.
  - Per layer, K^T and V are quantized to fp8e4 and AllGathered across all 8
    cores. Projection order K -> AG(K) -> V -> AG(V) -> Q so both collectives
    overlap projection compute.
  - Activations kept feature-major ([D, tok]); LayerNorm statistics via bf16
    ones-matmuls on pre-LN bf16 copies (cast on the Scalar engine, squares on
    Vector) accumulated into a reused PSUM slab; the rsqrt scale is
    Sqrt (ScalarE) + reciprocal_approx_fast (DVE) at full partition width so
    no broadcast matmul / Ln+Exp table thrash is needed; dummy activations
    preload ACT table sets off the critical path.
    (fp8 GEMM inputs for the projections/FFN were tried and reverted: each
    fp8xfp8 GEMM adds ~5% incoherent-sum noise -> ~10% final, over budget.)
  - Softmax: scores transposed (s^T[k, q]); exp() on the Scalar engine;
    denominator via a ones-column appended to V in the attn@V matmul, then
    reciprocal_approx_fast + a one-hot selector matmul to broadcast 1/den
    back over head dims (no DRAM round-trip).
"""

import numpy as np
import ml_dtypes

L, D, H, FF = 6, 1024, 16, 4096
DK = D // H          # 64
B, S = 2, 2048
NCORES = 8
TPB = S // NCORES    # 256 tokens per batch per core
T = 2 * TPB          # 512 local tokens per core (256 b0 + 256 b1)
DC = D // 128        # 8
FC = FF // 128       # 32
KC = S // 128        # 16 key chunks per batch
TC = T // 128        # 4
EPS = 1e-5
BF16 = ml_dtypes.bfloat16
FP8 = ml_dtypes.float8_e4m3
WS = 32.0            # fp8 weight pre-scale (host)
XS = 8.0             # fp8 activation pre-scale (on-chip)

_CACHE = {}


def _build_nc():
    import contextlib
    import concourse.bacc as bacc
    import concourse.mybir as mybir
    import concourse.tile as tile
    import concourse.bass as bass
    from concourse.bass import ts, ds

    f32 = mybir.dt.float32
    bf16 = mybir.dt.bfloat16
    fp8 = mybir.dt.float8e4
    AF = mybir.ActivationFunctionType
    OP = mybir.AluOpType
    DR = mybir.MatmulPerfMode.DoubleRow
    WXS = float(WS * XS)  # combined weight x activation fp8 pre-scale

    nc = bacc.Bacc(num_devices=NCORES)

    # ---- parameters -----------------------------------------------------
    x0T = nc.declare_dram_parameter("x0T", [D, T], f32, isOutput=False)
    wq = nc.declare_dram_parameter("wq", [L, D, D], bf16, isOutput=False)
    wk = nc.declare_dram_parameter("wk", [L, D, D], bf16, isOutput=False)
    wv = nc.declare_dram_parameter("wv", [L, D, D], bf16, isOutput=False)
    wo = nc.declare_dram_parameter("wo", [L, D, D], bf16, isOutput=False)
    w1 = nc.declare_dram_parameter("w1", [L, D, FF], bf16, isOutput=False)
    # W2 pre-packed on host: [L, mc(8), o(32), p(128), m(128)]
    w2p = nc.declare_dram_parameter("w2p", [L, DC, FC, 128, 128], bf16, isOutput=False)
    bq = nc.declare_dram_parameter("bq", [L, D], f32, isOutput=False)
    bk = nc.declare_dram_parameter("bk", [L, D], f32, isOutput=False)
    bvb = nc.declare_dram_parameter("bvb", [L, D], bf16, isOutput=False)
    bo = nc.declare_dram_parameter("bo", [L, D], f32, isOutput=False)
    b1 = nc.declare_dram_parameter("b1", [L, FF], f32, isOutput=False)
    b2 = nc.declare_dram_parameter("b2", [L, D], f32, isOutput=False)
    g1 = nc.declare_dram_parameter("g1", [L, D], f32, isOutput=False)
    be1 = nc.declare_dram_parameter("be1", [L, D], f32, isOutput=False)
    g2 = nc.declare_dram_parameter("g2", [L, D], f32, isOutput=False)
    be2 = nc.declare_dram_parameter("be2", [L, D], f32, isOutput=False)
    sel16p = nc.declare_dram_parameter("sel16p", [H, DC * 128], bf16,
                                       isOutput=False)
    outT = nc.declare_dram_parameter("outT", [D, T], f32, isOutput=True)

    groups8 = [list(range(NCORES))]

    with tile.TileContext(nc) as tc:
        ctx = contextlib.ExitStack()
        singles = ctx.enter_context(tc.tile_pool(name="singles", bufs=1))
        params = ctx.enter_context(tc.tile_pool(name="params", bufs=2))
        wpool = ctx.enter_context(tc.tile_pool(name="wpool", bufs=2))
        w1pool = ctx.enter_context(tc.tile_pool(name="w1pool", bufs=2))
        w2pool = ctx.enter_context(tc.tile_pool(name="w2pool", bufs=2))
        kgpool = ctx.enter_context(tc.tile_pool(name="kgpool", bufs=2))
        vgpool = ctx.enter_context(tc.tile_pool(name="vgpool", bufs=1))
        vgppool = ctx.enter_context(tc.tile_pool(name="vgppool", bufs=2))
        apool = ctx.enter_context(tc.tile_pool(name="apool", bufs=16))
        xbfpool = ctx.enter_context(tc.tile_pool(name="xbfpool", bufs=2))
        xprepool = ctx.enter_context(tc.tile_pool(name="xprepool", bufs=1))
        qkvpool = ctx.enter_context(tc.tile_pool(name="qkvpool", bufs=1))
        anfpool = ctx.enter_context(tc.tile_pool(name="anfpool", bufs=8))
        ao65pool = ctx.enter_context(tc.tile_pool(name="ao65pool", bufs=2))
        tmp = ctx.enter_context(tc.tile_pool(name="tmp", bufs=1))
        denp = ctx.enter_context(tc.tile_pool(name="denp", bufs=1))
        dram = ctx.enter_context(tc.tile_pool(name="dram", bufs=2, space="DRAM"))
        pscore = ctx.enter_context(tc.tile_pool(name="pscore", bufs=2, space="PSUM"))
        pav = ctx.enter_context(tc.tile_pool(name="pav", bufs=2, space="PSUM"))
        pmisc = ctx.enter_context(tc.tile_pool(name="pmisc", bufs=2, space="PSUM"))

        # ---- constants + resident state --------------------------------
        xT = singles.tile([128, DC, T], f32, name="xT")
        nc.sync.dma_start(out=xT, in_=x0T[:, :].rearrange("(c p) t -> p c t", p=128))
        ones_row_bf = singles.tile([1, 128], bf16, name="ones_row_bf")
        nc.vector.memset(ones_row_bf, 1.0)
        eps_col = singles.tile([128, 1], f32, name="eps_col")
        nc.vector.memset(eps_col, EPS)
        onesD_bf = singles.tile([128, 128], bf16, name="onesD_bf")
        nc.vector.memset(onesD_bf, 1.0 / D)
        # one-hot selector: sel16[h, j*128 + (h%2)*64 + d] = 1 iff h//2 == j
        sel16 = singles.tile([16, DC * 128], bf16, name="sel16")
        nc.sync.dma_start(out=sel16, in_=sel16p[:, :])
        junk = singles.tile([1, 2], f32, name="junk")
        xbf_cur = xbfpool.tile([128, DC, T], bf16, tag="xbf")
        for c in range(DC):
            nc.vector.tensor_copy(xbf_cur[:, c, :], xT[:, c, :])

        def emit_output_chunk(mc, ps, bias_col, stats_ps):
            """xT[mc] += ps + bias; stage stats for the next LN."""
            nc.vector.scalar_tensor_tensor(xT[:, mc, :], ps,
                                           bias_col, xT[:, mc, :],
                                           OP.add, OP.add)
            xpre = xprepool.tile([128, 512], bf16, tag="xpre", bufs=2, name="xpre")
            nc.scalar.copy(xpre, xT[:, mc, :])
            sq = xprepool.tile([128, 512], bf16, tag="sqb", bufs=2, name="sq")
            nc.vector.tensor_tensor(sq, xpre, xpre, OP.mult)
            nc.tensor.matmul(stats_ps[:, 0:512], onesD_bf, xpre,
                             start=(mc == 0), stop=(mc == DC - 1))
            nc.tensor.matmul(stats_ps[:, 512:1024], onesD_bf, sq,
                             start=(mc == 0), stop=(mc == DC - 1))

        def ln_finish(stats_ps, g_sb, be_sb, xbf_out):
            """x = LN(x) in place from stats; writes bf16 copy into xbf_out."""
            msq = tmp.tile([128, 512], f32, tag="tA", name="msq")
            nc.scalar.square(msq, stats_ps[:, 0:512])
            e2 = tmp.tile([128, 512], f32, tag="tB", name="e2")
            nc.vector.tensor_tensor(e2, stats_ps[:, 512:1024], msq, OP.subtract)
            sdv = tmp.tile([128, 512], f32, tag="tA", name="sdv")
            nc.scalar.activation(sdv, e2, AF.Sqrt, bias=eps_col)
            # preload the exp table set off the critical path (next exps /
            # relus are all servable by it; Copy/Square are in every set)
            nc.scalar.activation(junk[0:1, 0:1], eps_col[0:1, 0:1], AF.Exp)
            s_bc = tmp.tile([128, 512], f32, tag="tB", name="s_bc")
            with nc.allow_low_precision(reason="LN rsqrt, 18-bit recip ok"):
                nc.vector.reciprocal_approx_fast(out=s_bc, in_=sdv)
            for c in range(DC):
                t1 = tmp.tile([128, 512], f32, tag="t1", bufs=2, name="t1")
                nc.vector.tensor_tensor(t1, xT[:, c, :], stats_ps[:, 0:512],
                                        OP.subtract)
                nc.vector.tensor_tensor(t1, t1, s_bc, OP.mult)
                nc.vector.tensor_scalar(xT[:, c, :], t1, g_sb[:, c:c + 1],
                                        be_sb[:, c:c + 1], OP.mult, OP.add)
                nc.scalar.copy(xbf_out[:, c, :], xT[:, c, :])

        for l in range(L):
            # ---- per-layer params --------------------------------------
            pp = params.tile([128, 8, DC], f32, tag="pcol")
            for i_, t_src in enumerate([bq, bk, bo, b2, g1, be1, g2, be2]):
                nc.sync.dma_start(out=pp[:, i_, :],
                                  in_=t_src[l].rearrange("(c p) -> p c", p=128))
            bq_sb, bk_sb, bo_sb, b2_sb = pp[:, 0], pp[:, 1], pp[:, 2], pp[:, 3]
            g1_sb, be1_sb, g2_sb, be2_sb = pp[:, 4], pp[:, 5], pp[:, 6], pp[:, 7]
            b1_sb = params.tile([128, FC], f32, tag="pc32")
            nc.sync.dma_start(out=b1_sb, in_=b1[l].rearrange("(c p) -> p c", p=128))
            bv_row = params.tile([1, D], bf16, tag="bv_row", bufs=1)
            nc.sync.dma_start(out=bv_row, in_=bvb[l][None, :])

            xbf = xbf_cur

            # DRAM staging + gathered buffers (Shared, 8-rank); K is
            # gathered in halves so scores can start before V's gather and
            # the CC engine pipeline stays busy: order K-h1 -> V -> K-h2.
            kt_loc = dram.tile([D, T], fp8, tag="kt_loc")
            kt_g = dram.tile([NCORES * D, T], fp8, tag="kt_g",
                             addr_space="Shared")
            v_loc = dram.tile([T, D], fp8, tag="v_loc")
            v_g = dram.tile([NCORES * T, D], fp8, tag="v_g",
                            addr_space="Shared")

            kt_sb = qkvpool.tile([128, DC, T], fp8, tag="kt")

            # ---- K projection (fp8 DoubleRow) + single AllGather -------
            for kh in range(2):
                wk_h = wpool.tile([128, DC, 512], bf16, tag="w", name=f"wk_h{kh}")
                nc.sync.dma_start(
                    out=wk_h,
                    in_=wk[l][:, ds(kh * 512, 512)].rearrange("(c p) m -> p c m", p=128))
                for mcb in range(4):
                    mc = kh * 4 + mcb
                    ps = pmisc.tile([128, 512], f32, tag="pmisc")
                    for c in range(DC):
                        nc.tensor.matmul(ps, wk_h[:, c, ts(mcb, 128)], xbf[:, c, :],
                                         start=(c == 0), stop=(c == DC - 1))
                    nc.vector.tensor_scalar(kt_sb[:, mc, :], ps,
                                            bk_sb[:, mc:mc + 1], None, OP.add)
                    # stage this K chunk to DRAM immediately so the gather
                    # triggers right after the last chunk, not a full-tensor
                    # DMA later
                    nc.gpsimd.dma_start(
                        out=kt_loc[ds(mc * 128, 128), :]
                            .rearrange("(c p) t -> p c t", p=128),
                        in_=kt_sb[:, mc:mc + 1, :])
            nc.gpsimd.collective_compute(
                "AllGather", OP.bypass, replica_groups=groups8,
                ins=[kt_loc.opt()], outs=[kt_g.opt()])

            # ---- V projection (fp8 DoubleRow) + AllGather --------------
            # V weights stage in a w1pool slot (idle until the FFN1 prefetch)
            # so the wpool rotation lets wq's DMA start right after K proj
            # releases its buffer, instead of stalling behind wv.
            v_sb = qkvpool.tile([128, TC, D], fp8, tag="v")
            wv_b = w1pool.tile([128, DC, 1024], bf16, tag="w1b", name="wv_b")
            for nh in range(2):
                nc.sync.dma_start(
                    out=wv_b[:, :, ds(nh * 512, 512)],
                    in_=wv[l][:, ds(nh * 512, 512)].rearrange("(c p) m -> p c m", p=128))
                for t_ in range(TC):
                    ps = pmisc.tile([128, 512], f32, tag="pmisc")
                    for c in range(DC):
                        nc.tensor.matmul(ps, xbf[:, c, ts(t_, 128)],
                                         wv_b[:, c, ds(nh * 512, 512)],
                                         start=(c == 0), stop=False)
                    nc.tensor.matmul(ps, ones_row_bf, bv_row[:, ds(nh * 512, 512)],
                                     start=False, stop=True)
                    nc.vector.tensor_copy(v_sb[:, t_, ds(nh * 512, 512)], ps)
                    if nh == 1:
                        nc.gpsimd.dma_start(
                            out=v_loc[ds(t_ * 128, 128), :]
                                .rearrange("(c p) d -> p c d", p=128),
                            in_=v_sb[:, t_:t_ + 1, :])
            nc.gpsimd.collective_compute(
                "AllGather", OP.bypass, replica_groups=groups8,
                ins=[v_loc.opt()], outs=[v_g.opt()])

            # ---- Q projection (overlaps both AllGathers) ---------------
            qT = qkvpool.tile([128, DC, T], bf16, tag="qT")
            for qh in range(2):
                wq_h = wpool.tile([128, DC, 512], bf16, tag="w", name=f"wq_h{qh}")
                nc.sync.dma_start(
                    out=wq_h,
                    in_=wq[l][:, ds(qh * 512, 512)].rearrange("(c p) m -> p c m", p=128))
                for mcb in range(4):
                    mc = qh * 4 + mcb
                    ps = pmisc.tile([128, 512], f32, tag="pmisc")
                    for c in range(DC):
                        nc.tensor.matmul(ps, wq_h[:, c, ts(mcb, 128)], xbf[:, c, :],
                                         start=(c == 0), stop=(c == DC - 1))
                    nc.vector.tensor_scalar(qT[:, mc, :], ps,
                                            bq_sb[:, mc:mc + 1], None, OP.add)

            # prefetch Wo + first W1 halves during attention
            wo_h1 = wpool.tile([128, DC, 512], bf16, tag="w", name="wo_h1")
            nc.sync.dma_start(out=wo_h1,
                              in_=wo[l][:, 0:512].rearrange("(c p) m -> p c m", p=128))
            wo_h2 = wpool.tile([128, DC, 512], bf16, tag="w", name="wo_h2")
            nc.sync.dma_start(out=wo_h2,
                              in_=wo[l][:, 512:1024].rearrange("(c p) m -> p c m", p=128))
            w1_tiles = {}
            for b4 in range(2):
                w1b = w1pool.tile([128, DC, 1024], bf16, tag="w1b",
                                  name=f"w1b{b4}")
                w1_tiles[b4] = w1b
                nc.sync.dma_start(
                    out=w1b,
                    in_=w1[l][:, ds(b4 * 1024, 1024)]
                        .rearrange("(c p) m -> p c m", p=128))

            # ---- attention ---------------------------------------------
            # gathered layouts (all-static reads; every core needs both batches):
            #   kt_g rows: r(8) x pairchunk(4) x p(128); cols: [b0 256 | b1 256]
            #   v_g  rows: c32(32) x p(128) = rank r tokens [b0 256|b1 256]; cols D
            # batch-b key chunk kc (128 keys): rank r=kc//2, col c4 = 2*b+(kc%2)
            # v_g row chunk for (b, kc): c32 = (kc//2)*4 + 2*b + (kc%2)
            den_sb = denp.tile([16, 512], bf16, tag="den", name="den_sb")
            an_bf = xbfpool.tile([128, DC, T], bf16, tag="xbf")
            anf_tiles = []
            for j in range(DC):  # head pairs (2j, 2j+1)
                kgj = kgpool.tile([128, NCORES, T], fp8, tag="kg")
                nc.gpsimd.dma_start(
                    out=kgj,
                    in_=kt_g[:, :].rearrange("(r c p) t -> p r c t", p=128,
                                             c=8)[:, :, j, :])
                # per-pair V slice with ones columns: [128, 32, 130]
                vgp = vgppool.tile([128, 32, 130], fp8, tag="vgp")
                nc.gpsimd.dma_start(
                    out=vgp[:, :, 0:64],
                    in_=v_g[:, ds((2 * j) * 64, 64)]
                        .rearrange("(c p) w -> p c w", p=128))
                nc.gpsimd.dma_start(
                    out=vgp[:, :, 65:129],
                    in_=v_g[:, ds((2 * j + 1) * 64, 64)]
                        .rearrange("(c p) w -> p c w", p=128))
                nc.vector.memset(vgp[:, :, 64:65], 1.0)
                nc.vector.memset(vgp[:, :, 129:130], 1.0)

                at_tiles = []
                for kc in range(KC):
                    r, codd = kc // 2, kc % 2
                    pss = pscore.tile([128, 1024], f32, tag="pscore")
                    for b_ in range(2):
                        c4 = 2 * b_ + codd
                        nc.tensor.matmul(
                            pss[:, ds(b_ * 256, 256)],
                            kgj[0:64, r, ts(c4, 128)],
                            qT[0:64, j, ds(b_ * 256, 256)],
                            start=True, stop=True, tile_position=(0, 0))
                        nc.tensor.matmul(
                            pss[:, ds(512 + b_ * 256, 256)],
                            kgj[64:128, r, ts(c4, 128)],
                            qT[64:128, j, ds(b_ * 256, 256)],
                            start=True, stop=True, tile_position=(64, 0))
                    at = apool.tile([128, 1024], fp8, tag="attn")
                    nc.scalar.activation(at, pss, AF.Exp, scale=1.0 / 32.0)
                    at_tiles.append(at)
                anf = anfpool.tile([128, 512], bf16, tag="anf")
                for ab in range(2):
                    h = 2 * j + ab
                    for b_ in range(2):
                        pav_t = pav.tile([65, 512], f32, tag="pav")
                        for kc in range(KC):
                            c32 = (kc // 2) * 4 + 2 * b_ + (kc % 2)
                            nc.tensor.matmul(
                                pav_t[:, 0:256],
                                vgp[:, c32, ds(ab * 65, 65)],
                                at_tiles[kc][:, ds(ab * 512 + b_ * 256, 256)],
                                start=(kc == 0), stop=(kc == KC - 1))
                        ao65 = ao65pool.tile([65, 256], bf16, tag="ao65")
                        nc.vector.tensor_copy(ao65, pav_t[:, 0:256])
                        nc.sync.dma_start(
                            out=den_sb[h:h + 1, ds(b_ * 256, 256)],
                            in_=ao65[64:65, :])
                        nc.sync.dma_start(
                            out=anf[ds(ab * 64, 64), ds(b_ * 256, 256)],
                            in_=ao65[0:64, :])
                anf_tiles.append(anf)

            # preload the sqrt table set while attnV j=7 drains (off-path);
            # Copy/Square/Relu in between are servable by every set
            nc.scalar.activation(junk[0:1, 0:1], eps_col[0:1, 0:1], AF.Sqrt)

            # ---- normalize attn_out (1/den via approx recip + selector) -
            den_f_t = tmp.tile([128, 512], f32, tag="tA", name="den_f")
            den_f = den_f_t[0:16, :]
            nc.vector.tensor_copy(den_f, den_sb)
            den_r_t = tmp.tile([128, 512], f32, tag="tB", name="den_r")
            den_r = den_r_t[0:16, :]
            with nc.allow_low_precision(reason="softmax denom scale, ~0.4% ok"):
                nc.vector.reciprocal_approx_fast(out=den_r, in_=den_f)
            denr_bf = denp.tile([16, 512], bf16, tag="denrb", name="denr_bf")
            nc.vector.tensor_copy(denr_bf, den_r)
            for j in range(DC):
                rb_ps = pmisc.tile([128, 512], f32, tag="pmisc")
                nc.tensor.matmul(rb_ps, sel16[:, ts(j, 128)], denr_bf,
                                 start=True, stop=True)
                nc.vector.tensor_tensor(an_bf[:, j, :], anf_tiles[j], rb_ps,
                                        OP.mult)

            # ---- Wo + residual (+ LN1 stats staging) -------------------
            stats1 = pscore.tile([128, 1024], f32, tag="pscore", name="stats1")
            for mc in range(DC):
                wo_h = wo_h1 if mc < 4 else wo_h2
                ps = pmisc.tile([128, 512], f32, tag="pmisc")
                for c in range(DC):
                    nc.tensor.matmul(ps, wo_h[:, c, ts(mc % 4, 128)], an_bf[:, c, :],
                                     start=(c == 0), stop=(c == DC - 1))
                emit_output_chunk(mc, ps, bo_sb[:, mc:mc + 1], stats1)

            # ---- LN1 ----------------------------------------------------
            xbf2 = xbfpool.tile([128, DC, T], bf16, tag="xbf")
            ln_finish(stats1, g1_sb, be1_sb, xbf2)

            # ---- FFN ----------------------------------------------------
            ffT = vgpool.tile([128, FC, T], bf16, tag="vg")
            for b4 in range(4):
                if b4 + 2 < 4:
                    w1bn = w1pool.tile([128, DC, 1024], bf16, tag="w1b",
                                       name=f"w1b{b4 + 2}")
                    w1_tiles[b4 + 2] = w1bn
                    nc.sync.dma_start(
                        out=w1_tiles[b4 + 2],
                        in_=w1[l][:, ds((b4 + 2) * 1024, 1024)]
                            .rearrange("(c p) m -> p c m", p=128))
                w1b = w1_tiles[b4]
                for mcb in range(DC):
                    mc = b4 * 8 + mcb
                    ps = pmisc.tile([128, 512], f32, tag="pmisc")
                    for c in range(DC):
                        nc.tensor.matmul(ps, w1b[:, c, ts(mcb, 128)], xbf2[:, c, :],
                                         start=(c == 0), stop=(c == DC - 1))
                    nc.scalar.activation(ffT[:, mc, :], ps, AF.Relu,
                                         bias=b1_sb[:, mc:mc + 1])
            nc.scalar.activation(junk[0:1, 0:1], eps_col[0:1, 0:1], AF.Sqrt)
            stats2 = pscore.tile([128, 1024], f32, tag="pscore", name="stats2")
            for mc in range(DC):
                w2mc = w2pool.tile([128, FC, 128], bf16, tag="w2")
                nc.sync.dma_start(out=w2mc,
                                  in_=w2p[l, mc].rearrange("o p m -> p o m"))
                ps = pmisc.tile([128, 512], f32, tag="pmisc")
                for fc in range(FC):
                    nc.tensor.matmul(ps, w2mc[:, fc, :], ffT[:, fc, :],
                                     start=(fc == 0), stop=(fc == FC - 1))
                emit_output_chunk(mc, ps, b2_sb[:, mc:mc + 1], stats2)

            # ---- LN2 ----------------------------------------------------
            xbf_cur = xbfpool.tile([128, DC, T], bf16, tag="xbf")
            ln_finish(stats2, g2_sb, be2_sb, xbf_cur)

        # ---- output ----------------------------------------------------
        nc.sync.dma_start(out=outT[:, :].rearrange("(c p) t -> p c t", p=128), in_=xT)
        ctx.close()

    nc.compile()
    return nc


def _prepare_host(inputs):
    src = np.asarray(inputs["src"]).astype(np.int64)
    emb = np.asarray(inputs["emb"], dtype=np.float32)
    x = emb[src]                                    # [B, S, D] f32
    pos = np.arange(B, dtype=np.float32)[:, None]
    div = np.exp(np.arange(0, D, 2, dtype=np.float32) * (-np.log(10000.0) / D))
    pe = np.zeros((B, D), np.float32)
    pe[:, 0::2] = np.sin(pos / div)
    pe[:, 1::2] = np.cos(pos / div)
    x = x + pe[:, None, :]

    mask = np.asarray(inputs["src_mask"]).reshape(B, S)
    assert np.all(mask != 0), "kernel assumes all-ones src_mask (per problem spec)"

    f = np.float32
    shared = {
        "wq": np.ascontiguousarray(np.asarray(inputs["Wq"], f).astype(BF16)),
        "wk": np.ascontiguousarray(np.asarray(inputs["Wk"], f).astype(BF16)),
        "wv": np.ascontiguousarray(np.asarray(inputs["Wv"], f).astype(BF16)),
        "wo": np.ascontiguousarray(np.asarray(inputs["Wo"], f).astype(BF16)),
        "w1": np.ascontiguousarray(np.asarray(inputs["W1"], f).astype(BF16)),
        "w2p": np.ascontiguousarray(
            np.asarray(inputs["W2"], f).reshape(L, FC, 128, DC, 128)
            .transpose(0, 3, 1, 2, 4).astype(BF16)),
        "bq": np.ascontiguousarray(np.asarray(inputs["bq"], f)),
        "bk": np.ascontiguousarray(np.asarray(inputs["bk"], f)),
        "bvb": np.ascontiguousarray(np.asarray(inputs["bv"], f).astype(BF16)),
        "bo": np.ascontiguousarray(np.asarray(inputs["bo"], f)),
        "b1": np.ascontiguousarray(np.asarray(inputs["b1"], f)),
        "b2": np.ascontiguousarray(np.asarray(inputs["b2"], f)),
        "g1": np.ascontiguousarray(np.asarray(inputs["g1"], f)),
        "be1": np.ascontiguousarray(np.asarray(inputs["be1"], f)),
        "g2": np.ascontiguousarray(np.asarray(inputs["g2"], f)),
        "be2": np.ascontiguousarray(np.asarray(inputs["be2"], f)),
    }
    sel = np.zeros((H, DC * 128), np.float32)
    for h in range(H):
        o = (h // 2) * 128 + (h % 2) * 64
        sel[h, o:o + 64] = 1.0
    shared["sel16p"] = np.ascontiguousarray(sel.astype(BF16))
    in_maps = []
    for i in range(NCORES):
        t0 = i * TPB
        m = dict(shared)
        xi = np.concatenate([x[0, t0:t0 + TPB, :].T,
                             x[1, t0:t0 + TPB, :].T], axis=1)
        m["x0T"] = np.ascontiguousarray(xi.astype(np.float32))
        in_maps.append(m)
    return in_maps


def _run(in_maps, trace=False):
    from concourse.bass_utils import run_bass_kernel_spmd
    if "nc" not in _CACHE:
        _CACHE["nc"] = _build_nc()
    nc = _CACHE["nc"]
    res = run_bass_kernel_spmd(nc, in_maps, core_ids=list(range(NCORES)),
                               trace=trace)
    outs = res.results
    y = np.zeros((B, S, D), np.float32)
    for i in range(NCORES):
        t0 = i * TPB
        y[0, t0:t0 + TPB, :] = outs[i]["outT"][:, 0:TPB].T
        y[1, t0:t0 + TPB, :] = outs[i]["outT"][:, TPB:T].T
    return y, res


def kernel(**inputs) -> np.ndarray:
    in_maps = _prepare_host(inputs)
    y, _ = _run(in_maps, trace=False)
    return y


def kernel_traced(**inputs):
    """Same as kernel() but returns (output, BassKernelResults with profile)."""
    in_maps = _prepare_host(inputs)
    return _run(in_maps, trace=True)
